# revision 5
# baseline (speedup 1.0000x reference)
"""DigitCaps dynamic-routing kernel for 8x TRN2 NeuronCores, v2.

Data-parallel over batch (512 -> 64 per core). Per core, the routing loop is
restructured so u_hat (B*O*I*D) is never materialized, and the UV / softmax /
S stages are fused into ONE pipeline over i-chunks q (128 i's each):

  for q: per pair p: psA[(j,b),(d,i1)] = blk_v @ W_q   (PE, K=32=(o2,D))
         evict (ACT copy) -> mult x (DVE) -> d-tree (Pool+DVE) -> bval_q
         bridge transpose (PE) -> exp (ACT) -> Z,1/Z,xs (DVE)
         y = c*xs (DVE/Pool) -> S-matmul accumulate (PE)

squash(z) = z - 1 - gelu(z) + cos(z) - relu(z) is evaluated as a polynomial
  squash(z) = t*P(t) - 0.5*|z|,  t = z^2   (|z| <= ~0.8 in this problem)
entirely on DVE, so Exp is the only ACT table function in the kernel (one
table load, prefetched during the initial DMA).

b2 = b1 + UV(v1) = UV(v0+v1) by linearity, so each iteration is one UV pass.
"""

import numpy as np
import ml_dtypes

import concourse.bass as bass
import concourse.bacc as bacc
import concourse.mybir as mybir
from concourse.tile import TileContext
from concourse.bass_utils import run_bass_kernel_spmd

bf16 = ml_dtypes.bfloat16
F32 = mybir.dt.float32
BF = mybir.dt.bfloat16
AF = mybir.ActivationFunctionType
ALU = mybir.AluOpType
AX = mybir.AxisListType

B, O, I, D, d = 512, 10, 1152, 16, 8
BL = 64          # batch per core
NPAIR = 5        # o-pairs
NQ = 9           # i chunks of 128
NT = 72          # (q, d) tiles

import os
_B = lambda k, dflt: int(os.environ.get(k, dflt))

# squash(z) = t*P(t) - 0.5|z|, t=z^2, P quadratic (erf+cos series, |z|<1)
_C = 0.3989422804014327
SQ_P0 = -_C - 0.5
SQ_P1 = _C / 6.0 + 1.0 / 24.0
SQ_P2 = -_C / 40.0 - 1.0 / 720.0


def _squash(nc, pool, z_ap, shape, tag):
    """v = z - 1 - gelu(z) + cos(z) - relu(z) via polynomial; fp32, DVE only."""
    t = pool.tile(shape, F32, tag=f"{tag}_t")
    h = pool.tile(shape, F32, tag=f"{tag}_h")
    v = pool.tile(shape, F32, tag=f"{tag}_v")
    nc.vector.tensor_tensor(t[:], z_ap, z_ap, ALU.mult)
    nc.vector.tensor_scalar(h[:], t[:], SQ_P2, SQ_P1, ALU.mult, ALU.add)
    nc.vector.tensor_tensor(h[:], h[:], t[:], ALU.mult)
    nc.vector.scalar_tensor_tensor(h[:], h[:], SQ_P0, t[:], ALU.add, ALU.mult)
    nc.vector.tensor_single_scalar(t[:], z_ap, 0.0, ALU.abs_max)
    nc.vector.scalar_tensor_tensor(v[:], t[:], -0.5, h[:], ALU.mult, ALU.add)
    return v


def _body(nc, tc, x3_d, xq_d, w2_d, w1s_d, idb_d, id32_d, msk_d, out_d):
    TP = _B("TP", 1)      # tree L1 on Pool (1) or DVE (0)
    YPQ = _B("YPQ", 0)    # y-joint ops per q routed to Pool (0..2)
    XSP = _B("XSP", 1)    # xs on Pool (1) or DVE (0)
    ZADD = _B("ZADD", 1)  # Z via paired adds (1) or tensor_reduce (0)
    with (
        tc.tile_pool(name="const", bufs=1) as cpool,
        tc.tile_pool(name="work", bufs=1) as wpool,
        tc.tile_pool(name="small", bufs=2) as spool,
        tc.tile_pool(name="psMM", bufs=2, space="PSUM") as psMM_pool,
        tc.tile_pool(name="psA2", bufs=_B("PSA2", 2), space="PSUM") as psA2_pool,
        tc.tile_pool(name="psS", bufs=1, space="PSUM") as psS_pool,
    ):
        # one-time Exp table prefetch; overlaps the resident DMA loads
        dm0 = cpool.tile([1, 1], F32)
        nc.gpsimd.memset(dm0[:], 0.0)
        dummy = wpool.tile([1, 1], F32, tag="dummy")
        nc.scalar.activation(dummy[:], dm0[:], AF.Exp)

        # ---- resident loads (split for DMA parallelism) ----
        x3 = cpool.tile([128, NQ, d, BL], BF)
        xq = cpool.tile([128, NQ, d, 128], BF)
        w1s = cpool.tile([128, NT, 160], BF)
        idb = cpool.tile([128, 128], BF)
        id32 = cpool.tile([64, 64], F32)
        msk = cpool.tile([128, 2], F32)
        for q0 in range(0, NQ, 3):
            nc.sync.dma_start(x3[:, q0:q0 + 3], x3_d.ap()[:, q0:q0 + 3])
        for t0 in range(0, NT, 18):
            nc.sync.dma_start(w1s[:, t0:t0 + 18], w1s_d.ap()[:, t0:t0 + 18])
        for q0 in range(0, NQ, 3):
            nc.sync.dma_start(xq[:, q0:q0 + 3], xq_d.ap()[:, q0:q0 + 3])
        nc.sync.dma_start(idb[:], idb_d.ap())
        nc.sync.dma_start(id32[:], id32_d.ap())
        nc.sync.dma_start(msk[:], msk_d.ap())

        # ---- phase s0: s0[b,(o,D)] = sum_{i,d} x*W ----
        ps0 = psMM_pool.tile([BL, 160], F32, tag="mm")
        for t in range(NT):
            q, d_ = divmod(t, d)
            nc.tensor.matmul(ps0[:], x3[:, q, d_], w1s[:, t],
                             start=(t == 0), stop=(t == NT - 1))
        z0 = wpool.tile([BL, 160], F32, tag="z0")
        nc.scalar.mul(z0[:], ps0[:], 0.1)
        v0b = _squash(nc, wpool, z0[:], [BL, 160], "sq0")  # [64,160] b-world

        # transpose v0 per pair -> vT [32, (p,b)]
        vT = wpool.tile([32, NPAIR, BL], F32, tag="vT")
        for p in range(NPAIR):
            pst = psMM_pool.tile([32, BL], F32, tag="mm", name=f"pst0_{p}")
            nc.tensor.transpose(pst[:], v0b[:, 32 * p:32 * p + 32], id32[:])
            nc.scalar.copy(vT[:, p], pst[:])
        vsum = wpool.tile([32, NPAIR, BL], F32, tag="vsum")
        nc.vector.tensor_copy(vsum[:], vT[:])

        vfinal = None
        for it in range(2):
            vin = vT if it == 0 else vsum
            # block-diag lhsT [32, (p, 128)]: col j*64+b holds v for o=2p+j
            blk = wpool.tile([32, NPAIR, 128], BF, tag="blk")
            nc.vector.tensor_scalar(blk[:, :, 0:64], vin[:], msk[0:32, 0:1],
                                    None, ALU.mult)
            nc.vector.tensor_scalar(blk[:, :, 64:128], vin[:], msk[0:32, 1:2],
                                    None, ALU.mult)

            # prefetch W chunk for q=0
            w2t = spool.tile([32, NPAIR, 1024], BF, tag="w2t", bufs=2,
                             name=f"w2t_{it}_0")
            for p in range(NPAIR):
                nc.sync.dma_start(w2t[:, p], w2_d.ap()[:, p, 0])

            psS8 = psS_pool.tile([128, 512], F32, tag="psS8", name=f"psS8{it}")
            psS9 = psS_pool.tile([32, 128], F32, tag="psS9", name=f"psS9{it}")
            n8 = [0]  # matmul emit counters for start/stop flags

            def s_matmuls(y, q, dh4):
                for h in range(4):
                    t = q * d + 4 * dh4 + h
                    yf = y[:, h].rearrange("p o b -> p (o b)")
                    st = n8[0] == 0
                    sp = n8[0] == NT - 1
                    nc.tensor.matmul(psS8[:], w1s[:, t, 0:128], yf[:, 0:512],
                                     start=st, stop=sp)
                    nc.tensor.matmul(psS9[:], w1s[:, t, 128:160], yf[:, 512:640],
                                     start=st, stop=sp)
                    n8[0] += 1

            for q in range(NQ):
                w2cur = w2t
                if q + 1 < NQ:  # prefetch next q's W
                    w2t = spool.tile([32, NPAIR, 1024], BF, tag="w2t", bufs=2,
                                     name=f"w2t_{it}_{q + 1}")
                    for p in range(NPAIR):
                        nc.sync.dma_start(w2t[:, p], w2_d.ap()[:, p, q + 1])

                # ---- UV chunk: psA -> evict -> *x -> d-tree -> bval_q ----
                Aq = wpool.tile([128, NPAIR, 1024], BF, tag="Aq", bufs=2)
                for p in range(NPAIR):
                    psA = psA2_pool.tile([128, 1024], F32, tag="psA",
                                         name=f"psA{it}_{q}_{p}")
                    nc.tensor.matmul(psA[:, 0:512], blk[:, p],
                                     w2cur[:, p, 0:512], start=True, stop=True)
                    nc.tensor.matmul(psA[:, 512:1024], blk[:, p],
                                     w2cur[:, p, 512:1024], start=True, stop=True)
                    nc.scalar.copy(Aq[:, p], psA[:])
                    nc.vector.tensor_tensor(
                        Aq[:, p], Aq[:, p],
                        xq[:, q].rearrange("p a b -> p (a b)"), ALU.mult)
                bvalq = wpool.tile([128, NPAIR, 128], BF, tag="bvalq", bufs=2)
                eng1 = nc.gpsimd if TP else nc.vector
                eng1.tensor_tensor(Aq[:, :, 0:512], Aq[:, :, 0:512],
                                   Aq[:, :, 512:1024], ALU.add)
                nc.vector.tensor_tensor(Aq[:, :, 0:256], Aq[:, :, 0:256],
                                        Aq[:, :, 256:512], ALU.add)
                nc.vector.tensor_tensor(bvalq[:], Aq[:, :, 0:128],
                                        Aq[:, :, 128:256], ALU.add)

                # ---- bridge: transpose -> [i, (o,b)]; exp; Z; xs ----
                pst5 = psMM_pool.tile([128, 640], BF, tag="mm",
                                      name=f"pstb{it}_{q}")
                for p in range(NPAIR):
                    nc.tensor.transpose(pst5[:, 128 * p:128 * (p + 1)],
                                        bvalq[:, p], idb[:])
                cq = spool.tile([128, O, BL], BF, tag="cq", bufs=2)
                nc.scalar.activation(cq[:], pst5[:].rearrange(
                    "p (o b) -> p o b", o=O), AF.Exp)
                Z = spool.tile([128, BL], F32, tag="Z")
                if ZADD:
                    t1 = spool.tile([128, 5, BL], BF, tag="Zt1")
                    t2 = spool.tile([128, 2, BL], BF, tag="Zt2")
                    nc.vector.tensor_tensor(t1[:], cq[:, 0:5], cq[:, 5:10], ALU.add)
                    nc.vector.tensor_tensor(t2[:], t1[:, 0:2], t1[:, 2:4], ALU.add)
                    nc.vector.tensor_tensor(t2[:, 0], t2[:, 0], t2[:, 1], ALU.add)
                    nc.vector.tensor_tensor(Z[:], t2[:, 0], t1[:, 4], ALU.add)
                else:
                    nc.vector.tensor_reduce(Z[:], cq[:].rearrange("p o b -> p b o"),
                                            AX.X, ALU.add)
                rec = spool.tile([128, BL], BF, tag="rec")
                with nc.allow_low_precision(reason="1/Z at bf16; folded into xs"):
                    nc.vector.reciprocal(rec[:], Z[:])
                xsq = spool.tile([128, d, BL], BF, tag="xsq", bufs=2)
                xe = nc.gpsimd if XSP else nc.vector
                xe.tensor_tensor(xsq[:], x3[:, q],
                                 rec[:].unsqueeze(1).broadcast_to((128, d, BL)),
                                 ALU.mult)

                # ---- y = c*xs (4 d's per op) + S matmul accumulation ----
                ypool = []
                for dh4 in range(2):
                    y = spool.tile([128, 4, O, BL], BF, tag="y", bufs=_B("YB", 4))
                    use_pool = dh4 < YPQ
                    (nc.gpsimd if use_pool else nc.vector).tensor_tensor(
                        y[:], cq[:].unsqueeze(1).broadcast_to((128, 4, O, BL)),
                        xsq[:, 4 * dh4:4 * dh4 + 4].unsqueeze(2)
                        .broadcast_to((128, 4, O, BL)), ALU.mult)
                    if use_pool:
                        ypool.append((y, q, dh4))  # consume later
                    else:
                        s_matmuls(y, q, dh4)
                for y, qq, dh4 in ypool:
                    s_matmuls(y, qq, dh4)

            # ---- extract diag s, squash ----
            sT = wpool.tile([32, NPAIR, BL], F32, tag="sT")
            sTa = wpool.tile([32, NPAIR, BL], F32, tag="sTa")
            for p in range(4):
                nc.scalar.mul(sTa[:, p], psS8[32 * p:32 * p + 32, 128 * p:128 * p + 64],
                              msk[32 * p:32 * p + 32, 0:1])
                nc.scalar.mul(sT[:, p], psS8[32 * p:32 * p + 32, 128 * p + 64:128 * p + 128],
                              msk[32 * p:32 * p + 32, 1:2])
            nc.scalar.mul(sTa[:, 4], psS9[:, 0:64], msk[0:32, 0:1])
            nc.scalar.mul(sT[:, 4], psS9[:, 64:128], msk[0:32, 1:2])
            nc.vector.tensor_tensor(sT[:], sT[:], sTa[:], ALU.add)
            vnew = _squash(nc, wpool, sT[:], [32, NPAIR, BL], "sqi")
            if it == 0:
                nc.vector.tensor_tensor(vsum[:], vsum[:], vnew[:], ALU.add)
            else:
                vfinal = vnew

        # ---- output: vfinal [32=(o2,D), (p,b)] -> out[b, 2p+o2, D] ----
        out_ap = out_d.ap().rearrange("b (p o2) DD -> (o2 DD) p b", p=NPAIR, o2=2)
        for p in range(NPAIR):
            nc.sync.dma_start(out_ap[:, p], vfinal[:, p])


def build_program():
    nc = bacc.Bacc("TRN2", debug=False, target_bir_lowering=False)
    x3_d = nc.dram_tensor("x3", [128, NQ, d, BL], BF, kind="ExternalInput")
    xq_d = nc.dram_tensor("xq", [128, NQ, d, 128], BF, kind="ExternalInput")
    w2_d = nc.dram_tensor("w2", [32, NPAIR, NQ, d * 128], BF, kind="ExternalInput")
    w1s_d = nc.dram_tensor("w1s", [128, NT, 160], BF, kind="ExternalInput")
    idb_d = nc.dram_tensor("idb", [128, 128], BF, kind="ExternalInput")
    msk_d = nc.dram_tensor("msk", [128, 2], F32, kind="ExternalInput")
    id32_d = nc.dram_tensor("id32", [64, 64], F32, kind="ExternalInput")
    out_d = nc.dram_tensor("out", [BL, O, D], F32, kind="ExternalOutput")
    with TileContext(nc) as tc:
        _body(nc, tc, x3_d, xq_d, w2_d, w1s_d, idb_d, id32_d, msk_d, out_d)
    nc.compile()
    return nc


def host_prep_w(W):
    """W: [1,10,1152,16,8] fp32 -> (w2, w1s, idb, id32, msk) arrays."""
    Wb = W[0].astype(bf16)
    # w2[(o2,D), p, q, (d,i1)] = W[2p+o2, 128q+i1, D, d]
    w2 = np.ascontiguousarray(
        Wb.reshape(5, 2, NQ, 128, D, d).transpose(1, 4, 0, 2, 5, 3)
    ).reshape(32, NPAIR, NQ, d * 128)
    w1s = np.ascontiguousarray(
        Wb.reshape(5, 2, NQ, 128, D, d).transpose(3, 2, 5, 0, 1, 4)).reshape(128, NT, 160)
    idb = np.eye(128, dtype=bf16)
    id32 = np.eye(64, dtype=np.float32)
    msk = np.zeros((128, 2), np.float32)
    msk[:, 0] = np.tile(np.r_[np.ones(16), np.zeros(16)], 4)
    msk[:, 1] = 1.0 - msk[:, 0]
    return w2, w1s, idb, id32, msk


def host_prep_x(xc):
    """xc: [64, 1152, 8] fp32 -> (x3, xq)."""
    xb = xc.astype(bf16)
    x3 = np.ascontiguousarray(xb.reshape(BL, NQ, 128, d).transpose(2, 1, 3, 0))
    xr = np.ascontiguousarray(xb.reshape(BL, NQ, 128, d).transpose(0, 1, 3, 2))
    xqa = np.concatenate([xr, xr], axis=0)                  # [128, q, d, 128]
    return x3, xqa


_NC_CACHE = {}


def _get_nc():
    if "nc" not in _NC_CACHE:
        _NC_CACHE["nc"] = build_program()
    return _NC_CACHE["nc"]


def kernel(x, W):
    x = np.asarray(x, dtype=np.float32)
    W = np.asarray(W, dtype=np.float32)
    w2, w1s, idb, id32, msk = host_prep_w(W)
    in_maps = []
    for core in range(8):
        x3, xqa = host_prep_x(x[core * BL:(core + 1) * BL])
        in_maps.append({"x3": x3, "xq": xqa, "w2": w2, "w1s": w1s,
                        "idb": idb, "id32": id32, "msk": msk})
    nc = _get_nc()
    res = run_bass_kernel_spmd(nc, in_maps, list(range(8)))
    out = np.concatenate([res.results[i]["out"] for i in range(8)], axis=0)
    return out.astype(np.float32)


# revision 10
# speedup vs baseline: 1.1016x; 1.1016x over previous
"""DigitCaps dynamic-routing kernel for 8x TRN2 NeuronCores, v3.

Data-parallel over batch (512 -> 64 per core). Per core, the routing loop is
restructured so u_hat (B*O*I*D) is never materialized:

  s0      = 0.1 * x @ W            (PE, K=i*d contraction)
  v0      = squash(s0)
  bval_t  = UV(v_t):  A = W x_D v (PE per o-pair), bval = sum_d(A * x) (DVE)
  c_t     = softmax_o(bval_t)      (PE bridge transpose -> ACT exp -> DVE)
  s_t     = sum_i c*u_hat via y=c*x (DVE) then PE matmul over K=i per d
  v_t     = squash(s_t)

b2 = b1 + UV(v1) = UV(v0+v1) by linearity, so each iteration is one UV pass.

squash(z) = z - 1 - gelu(z) + cos(z) - relu(z) is evaluated as a polynomial
  squash(z) = t*P(t) - 0.5*|z|,  t = z^2   (|z| <= ~0.8 in this problem)
entirely on DVE, so Exp is the only ACT table function in the kernel (one
table load, prefetched during the initial DMA). The idle GPSIMD (Pool)
engine absorbs side-branch elementwise work: the d-reduction tree of early
o-pairs and part of the y=c*x products (consumed late by the S matmuls).
"""

import numpy as np
import ml_dtypes

import concourse.bass as bass
import concourse.bacc as bacc
import concourse.mybir as mybir
from concourse.tile import TileContext
from concourse.bass_utils import run_bass_kernel_spmd

bf16 = ml_dtypes.bfloat16
F32 = mybir.dt.float32
BF = mybir.dt.bfloat16
AF = mybir.ActivationFunctionType
ALU = mybir.AluOpType
AX = mybir.AxisListType

B, O, I, D, d = 512, 10, 1152, 16, 8
BL = 64          # batch per core
NPAIR = 5        # o-pairs
NQ = 9           # i chunks of 128
NT = 72          # (q, d) tiles

import os
_B = lambda k, dflt: int(os.environ.get(k, dflt))

# squash(z) = t*P(t) - 0.5|z|, t=z^2, P quadratic (erf+cos series, |z|<1)
_C = 0.3989422804014327
SQ_P0 = -_C - 0.5
SQ_P1 = _C / 6.0 + 1.0 / 24.0
SQ_P2 = -_C / 40.0 - 1.0 / 720.0


def _squash(nc, pool, z_ap, shape, tag):
    """v = z - 1 - gelu(z) + cos(z) - relu(z) via polynomial; fp32, DVE only."""
    t = pool.tile(shape, F32, tag=f"{tag}_t")
    h = pool.tile(shape, F32, tag=f"{tag}_h")
    v = pool.tile(shape, F32, tag=f"{tag}_v")
    nc.vector.tensor_tensor(t[:], z_ap, z_ap, ALU.mult)
    nc.vector.tensor_scalar(h[:], t[:], SQ_P2, SQ_P1, ALU.mult, ALU.add)
    nc.vector.tensor_tensor(h[:], h[:], t[:], ALU.mult)
    nc.vector.scalar_tensor_tensor(h[:], h[:], SQ_P0, t[:], ALU.add, ALU.mult)
    nc.vector.tensor_single_scalar(t[:], z_ap, 0.0, ALU.abs_max)
    nc.vector.scalar_tensor_tensor(v[:], t[:], -0.5, h[:], ALU.mult, ALU.add)
    return v


def _body(nc, tc, x3_d, xdi_d, w2_d, w1s_d, idb_d, id32_d, msk_d, out_d):
    PL = _B("PL", 3)      # pairs whose tree L1 runs on Pool (side branch)
    PL2 = _B("PL2", 2)    # pairs whose tree L2 runs on Pool
    YPQ = _B("YPQ", 1)    # y-joint ops per q routed to Pool (0..2)
    with (
        tc.tile_pool(name="const", bufs=1) as cpool,
        tc.tile_pool(name="work", bufs=1) as wpool,
        tc.tile_pool(name="small", bufs=2) as spool,
        tc.tile_pool(name="psMM", bufs=2, space="PSUM") as psMM_pool,
        tc.tile_pool(name="psA2", bufs=_B("PSA2", 2), space="PSUM") as psA2_pool,
        tc.tile_pool(name="psS", bufs=1, space="PSUM") as psS_pool,
    ):
        # one-time Exp table prefetch; overlaps the resident DMA loads
        dm0 = cpool.tile([1, 1], F32)
        nc.gpsimd.memset(dm0[:], 0.0)
        dummy = wpool.tile([1, 1], F32, tag="dummy")
        nc.scalar.activation(dummy[:], dm0[:], AF.Exp)

        # ---- resident loads; x3/w1s first (s0 consumes them in t order) ----
        x3 = cpool.tile([128, NQ, d, BL], BF)
        xdi = cpool.tile([128, d, I], BF)
        w1s = cpool.tile([128, NT, 160], BF)
        idb = cpool.tile([128, 128], BF)
        id32 = cpool.tile([64, 64], F32)
        msk = cpool.tile([128, 2], F32)
        nc.sync.dma_start(id32[:], id32_d.ap())
        nc.sync.dma_start(msk[:], msk_d.ap())
        for g in range(3):
            nc.sync.dma_start(x3[:, 3 * g:3 * g + 3], x3_d.ap()[:, 3 * g:3 * g + 3])
            nc.sync.dma_start(w1s[:, 24 * g:24 * g + 24],
                              w1s_d.ap()[:, 24 * g:24 * g + 24])
        nc.sync.dma_start(idb[:], idb_d.ap())
        for d0 in range(0, d, 2):
            nc.sync.dma_start(xdi[:, d0:d0 + 2], xdi_d.ap()[:, d0:d0 + 2])

        # ---- phase s0: s0[b,(o,D)] = sum_{i,d} x*W ----
        ps0 = psMM_pool.tile([BL, 160], F32, tag="mm")
        for t in range(NT):
            q, d_ = divmod(t, d)
            nc.tensor.matmul(ps0[:], x3[:, q, d_], w1s[:, t],
                             start=(t == 0), stop=(t == NT - 1))
        z0 = wpool.tile([BL, 160], F32, tag="z0")
        nc.scalar.mul(z0[:], ps0[:], 0.1)
        v0b = _squash(nc, wpool, z0[:], [BL, 160], "sq0")  # [64,160] b-world

        # transpose v0 per pair -> vT [32, (p,b)]
        vT = wpool.tile([32, NPAIR, BL], F32, tag="vT")
        for p in range(NPAIR):
            pst = psMM_pool.tile([32, BL], F32, tag="mm", name=f"pst0_{p}")
            nc.tensor.transpose(pst[:], v0b[:, 32 * p:32 * p + 32], id32[:])
            nc.scalar.copy(vT[:, p], pst[:])
        vsum = wpool.tile([32, NPAIR, BL], F32, tag="vsum")
        nc.vector.tensor_copy(vsum[:], vT[:])

        vfinal = None
        for it in range(2):
            vin = vT if it == 0 else vsum
            # block-diag lhsT [32, (p, 128)]: col j*64+b holds v for o=2p+j
            blk = wpool.tile([32, NPAIR, 128], BF, tag="blk")
            nc.vector.tensor_scalar(blk[:, :, 0:64], vin[:], msk[0:32, 0:1],
                                    None, ALU.mult)
            nc.vector.tensor_scalar(blk[:, :, 64:128], vin[:], msk[0:32, 1:2],
                                    None, ALU.mult)

            # ---- UV: A + evict + mult + d-tree -> bval [128,(p,i)] ----
            bval = wpool.tile([128, NPAIR, I], BF, tag="bval")
            for p in range(NPAIR):
                w2t = spool.tile([32, d * I], BF, tag="w2t", bufs=2)
                for d0 in range(0, d, 2):
                    nc.sync.dma_start(
                        w2t[:, d0 * I:(d0 + 2) * I],
                        w2_d.ap()[:, p, d0:d0 + 2].rearrange("p a b -> p (a b)"))
                A_sb = wpool.tile([128, d * I], BF, tag="A_sb", bufs=_B("ASB", 3))
                FD = _B("FD", 0)  # chunks per pair with DVE-fused evict+mult
                xdi_f = xdi[:].rearrange("p a b -> p (a b)")
                for n in range(9):
                    psA = psA2_pool.tile([128, 1024], F32, tag="psA",
                                         name=f"psA{it}_{p}_{n}")
                    nc.tensor.matmul(psA[:, 0:512], blk[:, p],
                                     w2t[:, 1024 * n:1024 * n + 512],
                                     start=True, stop=True)
                    nc.tensor.matmul(psA[:, 512:1024], blk[:, p],
                                     w2t[:, 1024 * n + 512:1024 * (n + 1)],
                                     start=True, stop=True)
                    if n < FD:  # fused PSUM-read eviction + x-product on DVE
                        nc.vector.tensor_tensor(
                            A_sb[:, 1024 * n:1024 * (n + 1)], psA[:],
                            xdi_f[:, 1024 * n:1024 * (n + 1)], ALU.mult)
                    else:
                        nc.scalar.copy(A_sb[:, 1024 * n:1024 * (n + 1)], psA[:])
                # in-place: prod and the d-reduction tree reuse A_sb columns
                if FD < 9:
                    nc.vector.tensor_tensor(A_sb[:, 1024 * FD:], A_sb[:, 1024 * FD:],
                                            xdi_f[:, 1024 * FD:], ALU.mult)
                e1 = nc.gpsimd if p < PL else nc.vector
                e2 = nc.gpsimd if p < PL2 else nc.vector
                e1.tensor_tensor(A_sb[:, 0:4 * I], A_sb[:, 0:4 * I],
                                 A_sb[:, 4 * I:8 * I], ALU.add)
                e2.tensor_tensor(A_sb[:, 0:2 * I], A_sb[:, 0:2 * I],
                                 A_sb[:, 2 * I:4 * I], ALU.add)
                nc.vector.tensor_tensor(bval[:, p], A_sb[:, 0:I],
                                        A_sb[:, I:2 * I], ALU.add)

            # ---- softmax+S pipeline over q ----
            psS8 = psS_pool.tile([128, 512], F32, tag="psS8", name=f"psS8{it}")
            psS9 = psS_pool.tile([32, 128], F32, tag="psS9", name=f"psS9{it}")
            n8 = [0]  # matmul emit counter for start/stop flags

            def s_matmuls(y, q, d0, nd):
                for h in range(nd):
                    t = q * d + d0 + h
                    yf = y[:, h].rearrange("p o b -> p (o b)")
                    st = n8[0] == 0
                    sp = n8[0] == NT - 1
                    nc.tensor.matmul(psS8[:], w1s[:, t, 0:128], yf[:, 0:512],
                                     start=st, stop=sp)
                    nc.tensor.matmul(psS9[:], w1s[:, t, 128:160], yf[:, 512:640],
                                     start=st, stop=sp)
                    n8[0] += 1

            ydefer = []
            for q in range(NQ):
                pst5 = psMM_pool.tile([128, 640], BF, tag="mm",
                                      name=f"pstb{it}_{q}")
                for p in range(NPAIR):
                    nc.tensor.transpose(pst5[:, 128 * p:128 * (p + 1)],
                                        bval[:, p, 128 * q:128 * (q + 1)], idb[:])
                cq = spool.tile([128, O, BL], BF, tag="cq", bufs=3)
                nc.scalar.activation(cq[:], pst5[:].rearrange(
                    "p (o b) -> p o b", o=O), AF.Exp)
                Z = spool.tile([128, BL], F32, tag="Z")
                nc.vector.tensor_reduce(Z[:], cq[:].rearrange("p o b -> p b o"),
                                        AX.X, ALU.add)
                rec = spool.tile([128, BL], BF, tag="rec")
                with nc.allow_low_precision(reason="1/Z at bf16; folded into xs"):
                    nc.vector.reciprocal(rec[:], Z[:])
                xsq = spool.tile([128, d, BL], BF, tag="xsq", bufs=2)
                nc.vector.tensor_tensor(
                    xsq[:], x3[:, q],
                    rec[:].unsqueeze(1).broadcast_to((128, d, BL)), ALU.mult)

                # y split: d-slices 0..5 on DVE, 6..7 on Pool (consumed a q late)
                DP = _B("DP", 6)  # d-slices on DVE; rest on Pool
                yD = spool.tile([128, DP, O, BL], BF, tag="yD", bufs=2)
                nc.vector.tensor_tensor(
                    yD[:], cq[:].unsqueeze(1).broadcast_to((128, DP, O, BL)),
                    xsq[:, 0:DP].unsqueeze(2).broadcast_to((128, DP, O, BL)),
                    ALU.mult)
                if DP < d:
                    dP = d - DP
                    yP = spool.tile([128, dP, O, BL], BF, tag="yP", bufs=3)
                    (nc.gpsimd if YPQ else nc.vector).tensor_tensor(
                        yP[:], cq[:].unsqueeze(1).broadcast_to((128, dP, O, BL)),
                        xsq[:, DP:d].unsqueeze(2).broadcast_to((128, dP, O, BL)),
                        ALU.mult)
                s_matmuls(yD, q, 0, DP)
                if DP < d:
                    ydefer.append((yP, q, DP, d - DP))
                    if len(ydefer) > 1:
                        s_matmuls(*ydefer.pop(0))
            while ydefer:
                s_matmuls(*ydefer.pop(0))

            # ---- extract diag s, squash ----
            sT = wpool.tile([32, NPAIR, BL], F32, tag="sT")
            sTa = wpool.tile([32, NPAIR, BL], F32, tag="sTa")
            for p in range(4):
                nc.scalar.mul(sTa[:, p], psS8[32 * p:32 * p + 32, 128 * p:128 * p + 64],
                              msk[32 * p:32 * p + 32, 0:1])
                nc.scalar.mul(sT[:, p], psS8[32 * p:32 * p + 32, 128 * p + 64:128 * p + 128],
                              msk[32 * p:32 * p + 32, 1:2])
            nc.scalar.mul(sTa[:, 4], psS9[:, 0:64], msk[0:32, 0:1])
            nc.scalar.mul(sT[:, 4], psS9[:, 64:128], msk[0:32, 1:2])
            nc.vector.tensor_tensor(sT[:], sT[:], sTa[:], ALU.add)
            vnew = _squash(nc, wpool, sT[:], [32, NPAIR, BL], "sqi")
            if it == 0:
                nc.vector.tensor_tensor(vsum[:], vsum[:], vnew[:], ALU.add)
            else:
                vfinal = vnew

        # ---- output: vfinal [32=(o2,D), (p,b)] -> out[b, 2p+o2, D] ----
        out_ap = out_d.ap().rearrange("b (p o2) DD -> (o2 DD) p b", p=NPAIR, o2=2)
        for p in range(NPAIR):
            nc.sync.dma_start(out_ap[:, p], vfinal[:, p])


def build_program():
    nc = bacc.Bacc("TRN2", debug=False, target_bir_lowering=False)
    x3_d = nc.dram_tensor("x3", [128, NQ, d, BL], BF, kind="ExternalInput")
    xdi_d = nc.dram_tensor("xdi", [128, d, I], BF, kind="ExternalInput")
    w2_d = nc.dram_tensor("w2", [32, NPAIR, d, I], BF, kind="ExternalInput")
    w1s_d = nc.dram_tensor("w1s", [128, NT, 160], BF, kind="ExternalInput")
    idb_d = nc.dram_tensor("idb", [128, 128], BF, kind="ExternalInput")
    msk_d = nc.dram_tensor("msk", [128, 2], F32, kind="ExternalInput")
    id32_d = nc.dram_tensor("id32", [64, 64], F32, kind="ExternalInput")
    out_d = nc.dram_tensor("out", [BL, O, D], F32, kind="ExternalOutput")
    with TileContext(nc) as tc:
        _body(nc, tc, x3_d, xdi_d, w2_d, w1s_d, idb_d, id32_d, msk_d, out_d)
    nc.compile()
    return nc


def host_prep_w(W):
    """W: [1,10,1152,16,8] fp32 -> (w2, w1s, idb, id32, msk) arrays."""
    Wb = W[0].astype(bf16)
    w2 = np.ascontiguousarray(
        Wb.reshape(5, 2, I, D, d).transpose(1, 3, 0, 4, 2)).reshape(32, NPAIR, d, I)
    w1s = np.ascontiguousarray(
        Wb.reshape(5, 2, NQ, 128, D, d).transpose(3, 2, 5, 0, 1, 4)).reshape(128, NT, 160)
    idb = np.eye(128, dtype=bf16)
    id32 = np.eye(64, dtype=np.float32)
    msk = np.zeros((128, 2), np.float32)
    msk[:, 0] = np.tile(np.r_[np.ones(16), np.zeros(16)], 4)
    msk[:, 1] = 1.0 - msk[:, 0]
    return w2, w1s, idb, id32, msk


def host_prep_x(xc):
    """xc: [64, 1152, 8] fp32 -> (x3, xdi)."""
    xb = xc.astype(bf16)
    x3 = np.ascontiguousarray(xb.reshape(BL, NQ, 128, d).transpose(2, 1, 3, 0))
    xd = np.ascontiguousarray(xb.transpose(0, 2, 1))        # [64, 8, 1152]
    xdi = np.concatenate([xd, xd], axis=0)                  # [128, 8, 1152]
    return x3, xdi


_NC_CACHE = {}


def _get_nc():
    if "nc" not in _NC_CACHE:
        _NC_CACHE["nc"] = build_program()
    return _NC_CACHE["nc"]


def kernel(x, W):
    x = np.asarray(x, dtype=np.float32)
    W = np.asarray(W, dtype=np.float32)
    w2, w1s, idb, id32, msk = host_prep_w(W)
    in_maps = []
    for core in range(8):
        x3, xdi = host_prep_x(x[core * BL:(core + 1) * BL])
        in_maps.append({"x3": x3, "xdi": xdi, "w2": w2, "w1s": w1s,
                        "idb": idb, "id32": id32, "msk": msk})
    nc = _get_nc()
    res = run_bass_kernel_spmd(nc, in_maps, list(range(8)))
    out = np.concatenate([res.results[i]["out"] for i in range(8)], axis=0)
    return out.astype(np.float32)


# revision 11
# speedup vs baseline: 1.1721x; 1.0640x over previous
"""DigitCaps dynamic-routing kernel for 8x TRN2 NeuronCores, v3.

Data-parallel over batch (512 -> 64 per core). Per core, the routing loop is
restructured so u_hat (B*O*I*D) is never materialized:

  s0      = 0.1 * x @ W            (PE, K=i*d contraction)
  v0      = squash(s0)
  bval_t  = UV(v_t):  A = W x_D v (PE per o-pair), bval = sum_d(A * x) (DVE)
  c_t     = softmax_o(bval_t)      (PE bridge transpose -> ACT exp -> DVE)
  s_t     = sum_i c*u_hat via y=c*x (DVE) then PE matmul over K=i per d
  v_t     = squash(s_t)

b2 = b1 + UV(v1) = UV(v0+v1) by linearity, so each iteration is one UV pass.

squash(z) = z - 1 - gelu(z) + cos(z) - relu(z) is evaluated as a polynomial
  squash(z) = t*P(t) - 0.5*|z|,  t = z^2   (|z| <= ~0.8 in this problem)
entirely on DVE, so Exp is the only ACT table function in the kernel (one
table load, prefetched during the initial DMA). The idle GPSIMD (Pool)
engine absorbs side-branch elementwise work: the d-reduction tree of early
o-pairs and part of the y=c*x products (consumed late by the S matmuls).
"""

import numpy as np
import ml_dtypes

import concourse.bass as bass
import concourse.bacc as bacc
import concourse.mybir as mybir
from concourse.tile import TileContext
from concourse.bass_utils import run_bass_kernel_spmd

bf16 = ml_dtypes.bfloat16
F32 = mybir.dt.float32
BF = mybir.dt.bfloat16
AF = mybir.ActivationFunctionType
ALU = mybir.AluOpType
AX = mybir.AxisListType

B, O, I, D, d = 512, 10, 1152, 16, 8
BL = 64          # batch per core
NPAIR = 5        # o-pairs
NQ = 9           # i chunks of 128
NT = 72          # (q, d) tiles

import os
_B = lambda k, dflt: int(os.environ.get(k, dflt))

# squash(z) = t*P(t) - 0.5|z|, t=z^2, P quadratic (erf+cos series, |z|<1)
_C = 0.3989422804014327
SQ_P0 = -_C - 0.5
SQ_P1 = _C / 6.0 + 1.0 / 24.0
SQ_P2 = -_C / 40.0 - 1.0 / 720.0


def _squash(nc, pool, z_ap, shape, tag):
    """v = z - 1 - gelu(z) + cos(z) - relu(z) via polynomial; fp32, DVE only."""
    t = pool.tile(shape, F32, tag=f"{tag}_t")
    h = pool.tile(shape, F32, tag=f"{tag}_h")
    v = pool.tile(shape, F32, tag=f"{tag}_v")
    nc.vector.tensor_tensor(t[:], z_ap, z_ap, ALU.mult)
    nc.vector.tensor_scalar(h[:], t[:], SQ_P2, SQ_P1, ALU.mult, ALU.add)
    nc.vector.tensor_tensor(h[:], h[:], t[:], ALU.mult)
    nc.vector.scalar_tensor_tensor(h[:], h[:], SQ_P0, t[:], ALU.add, ALU.mult)
    nc.vector.tensor_single_scalar(t[:], z_ap, 0.0, ALU.abs_max)
    nc.vector.scalar_tensor_tensor(v[:], t[:], -0.5, h[:], ALU.mult, ALU.add)
    return v


def _body(nc, tc, x3_d, xdi_d, w2_d, w1s_d, idb_d, id32_d, msk_d, out_d):
    PL = _B("PL", 3)      # pairs whose tree L1 runs on Pool (side branch)
    PL2 = _B("PL2", 2)    # pairs whose tree L2 runs on Pool
    YPQ = _B("YPQ", 1)    # y-joint ops per q routed to Pool (0..2)
    with (
        tc.tile_pool(name="const", bufs=1) as cpool,
        tc.tile_pool(name="work", bufs=1) as wpool,
        tc.tile_pool(name="small", bufs=2) as spool,
        tc.tile_pool(name="psMM", bufs=2, space="PSUM") as psMM_pool,
        tc.tile_pool(name="psA2", bufs=_B("PSA2", 2), space="PSUM") as psA2_pool,
        tc.tile_pool(name="psS", bufs=1, space="PSUM") as psS_pool,
    ):
        # one-time Exp table prefetch; overlaps the resident DMA loads
        dm0 = cpool.tile([1, 1], F32)
        nc.gpsimd.memset(dm0[:], 0.0)
        dummy = wpool.tile([1, 1], F32, tag="dummy")
        nc.scalar.activation(dummy[:], dm0[:], AF.Exp)

        # ---- resident loads; x3/w1s first (s0 consumes them in t order) ----
        x3 = cpool.tile([128, NQ, d, BL], BF)
        xdi = cpool.tile([128, d, I], BF)
        w1s = cpool.tile([128, NT, 160], BF)
        idb = cpool.tile([128, 128], BF)
        id32 = cpool.tile([64, 64], F32)
        msk = cpool.tile([128, 2], F32)
        nc.sync.dma_start(id32[:], id32_d.ap())
        nc.sync.dma_start(msk[:], msk_d.ap())
        for g in range(3):
            nc.sync.dma_start(x3[:, 3 * g:3 * g + 3], x3_d.ap()[:, 3 * g:3 * g + 3])
            nc.sync.dma_start(w1s[:, 24 * g:24 * g + 24],
                              w1s_d.ap()[:, 24 * g:24 * g + 24])
        nc.sync.dma_start(idb[:], idb_d.ap())
        for d0 in range(0, d, 2):
            nc.sync.dma_start(xdi[:, d0:d0 + 2], xdi_d.ap()[:, d0:d0 + 2])

        # ---- phase s0: s0[b,(o,D)] = sum_{i,d} x*W ----
        ps0 = psMM_pool.tile([BL, 160], F32, tag="mm")
        for t in range(NT):
            q, d_ = divmod(t, d)
            nc.tensor.matmul(ps0[:], x3[:, q, d_], w1s[:, t],
                             start=(t == 0), stop=(t == NT - 1))
        z0 = wpool.tile([BL, 160], F32, tag="z0")
        nc.scalar.mul(z0[:], ps0[:], 0.1)
        v0b = _squash(nc, wpool, z0[:], [BL, 160], "sq0")  # [64,160] b-world

        # transpose v0 per pair -> vT [32, (p,b)]
        vT = wpool.tile([32, NPAIR, BL], F32, tag="vT")
        for p in range(NPAIR):
            pst = psMM_pool.tile([32, BL], F32, tag="mm", name=f"pst0_{p}")
            nc.tensor.transpose(pst[:], v0b[:, 32 * p:32 * p + 32], id32[:])
            nc.scalar.copy(vT[:, p], pst[:])
        vsum = wpool.tile([32, NPAIR, BL], F32, tag="vsum")
        nc.vector.tensor_copy(vsum[:], vT[:])

        vfinal = None
        for it in range(2):
            vin = vT if it == 0 else vsum
            # block-diag lhsT [32, (p, 128)]: col j*64+b holds v for o=2p+j
            blk = wpool.tile([32, NPAIR, 128], BF, tag="blk")
            nc.vector.tensor_scalar(blk[:, :, 0:64], vin[:], msk[0:32, 0:1],
                                    None, ALU.mult)
            nc.vector.tensor_scalar(blk[:, :, 64:128], vin[:], msk[0:32, 1:2],
                                    None, ALU.mult)

            # ---- UV: A + evict + mult + d-tree -> bval [128,(p,i)] ----
            bval = wpool.tile([128, NPAIR, I], BF, tag="bval")
            for p in range(NPAIR):
                w2t = spool.tile([32, d * I], BF, tag="w2t", bufs=2)
                for d0 in range(0, d, 2):
                    nc.sync.dma_start(
                        w2t[:, d0 * I:(d0 + 2) * I],
                        w2_d.ap()[:, p, d0:d0 + 2].rearrange("p a b -> p (a b)"))
                A_sb = wpool.tile([128, d * I], BF, tag="A_sb", bufs=_B("ASB", 3))
                FD = _B("FD", 0)  # chunks per pair with DVE-fused evict+mult
                xdi_f = xdi[:].rearrange("p a b -> p (a b)")
                for n in range(9):
                    psA = psA2_pool.tile([128, 1024], F32, tag="psA",
                                         name=f"psA{it}_{p}_{n}")
                    nc.tensor.matmul(psA[:, 0:512], blk[:, p],
                                     w2t[:, 1024 * n:1024 * n + 512],
                                     start=True, stop=True)
                    nc.tensor.matmul(psA[:, 512:1024], blk[:, p],
                                     w2t[:, 1024 * n + 512:1024 * (n + 1)],
                                     start=True, stop=True)
                    if n < FD:  # fused PSUM-read eviction + x-product on DVE
                        nc.vector.tensor_tensor(
                            A_sb[:, 1024 * n:1024 * (n + 1)], psA[:],
                            xdi_f[:, 1024 * n:1024 * (n + 1)], ALU.mult)
                    else:
                        nc.scalar.copy(A_sb[:, 1024 * n:1024 * (n + 1)], psA[:])
                # in-place: prod and the d-reduction tree reuse A_sb columns
                if FD < 9:
                    nc.vector.tensor_tensor(A_sb[:, 1024 * FD:], A_sb[:, 1024 * FD:],
                                            xdi_f[:, 1024 * FD:], ALU.mult)
                e1 = nc.gpsimd if p < PL else nc.vector
                e2 = nc.gpsimd if p < PL2 else nc.vector
                e1.tensor_tensor(A_sb[:, 0:4 * I], A_sb[:, 0:4 * I],
                                 A_sb[:, 4 * I:8 * I], ALU.add)
                e2.tensor_tensor(A_sb[:, 0:2 * I], A_sb[:, 0:2 * I],
                                 A_sb[:, 2 * I:4 * I], ALU.add)
                nc.vector.tensor_tensor(bval[:, p], A_sb[:, 0:I],
                                        A_sb[:, I:2 * I], ALU.add)

            # ---- softmax+S pipeline over q ----
            psS8 = psS_pool.tile([128, 512], F32, tag="psS8", name=f"psS8{it}")
            psS9 = psS_pool.tile([32, 128], F32, tag="psS9", name=f"psS9{it}")
            n8 = [0]  # matmul emit counter for start/stop flags

            def s_matmuls(y, q, d0, nd):
                for h in range(nd):
                    t = q * d + d0 + h
                    yf = y[:, h].rearrange("p o b -> p (o b)")
                    st = n8[0] == 0
                    sp = n8[0] == NT - 1
                    nc.tensor.matmul(psS8[:], w1s[:, t, 0:128], yf[:, 0:512],
                                     start=st, stop=sp)
                    nc.tensor.matmul(psS9[:], w1s[:, t, 128:160], yf[:, 512:640],
                                     start=st, stop=sp)
                    n8[0] += 1

            ydefer = []
            for q in range(NQ):
                pst5 = psMM_pool.tile([128, 640], BF, tag="mm",
                                      name=f"pstb{it}_{q}")
                for p in range(NPAIR):
                    nc.tensor.transpose(pst5[:, 128 * p:128 * (p + 1)],
                                        bval[:, p, 128 * q:128 * (q + 1)], idb[:])
                cq = spool.tile([128, O, BL], BF, tag="cq", bufs=3)
                nc.scalar.activation(cq[:], pst5[:].rearrange(
                    "p (o b) -> p o b", o=O), AF.Exp)
                Z = spool.tile([128, BL], F32, tag="Z")
                nc.vector.tensor_reduce(Z[:], cq[:].rearrange("p o b -> p b o"),
                                        AX.X, ALU.add)
                rec = spool.tile([128, BL], BF, tag="rec")
                with nc.allow_low_precision(reason="1/Z at bf16; folded into xs"):
                    nc.vector.reciprocal(rec[:], Z[:])
                xsq = spool.tile([128, d, BL], BF, tag="xsq", bufs=2)
                nc.vector.tensor_tensor(
                    xsq[:], x3[:, q],
                    rec[:].unsqueeze(1).broadcast_to((128, d, BL)), ALU.mult)

                # y split: d-slices 0..5 on DVE, 6..7 on Pool (consumed a q late)
                DP = _B("DP", 6)  # d-slices on DVE; rest on Pool
                yD = spool.tile([128, DP, O, BL], BF, tag="yD", bufs=2)
                nc.vector.tensor_tensor(
                    yD[:], cq[:].unsqueeze(1).broadcast_to((128, DP, O, BL)),
                    xsq[:, 0:DP].unsqueeze(2).broadcast_to((128, DP, O, BL)),
                    ALU.mult)
                if DP < d:
                    dP = d - DP
                    yP = spool.tile([128, dP, O, BL], BF, tag="yP", bufs=3)
                    (nc.gpsimd if YPQ else nc.vector).tensor_tensor(
                        yP[:], cq[:].unsqueeze(1).broadcast_to((128, dP, O, BL)),
                        xsq[:, DP:d].unsqueeze(2).broadcast_to((128, dP, O, BL)),
                        ALU.mult)
                s_matmuls(yD, q, 0, DP)
                if DP < d:
                    ydefer.append((yP, q, DP, d - DP))
                    if len(ydefer) > 1:
                        s_matmuls(*ydefer.pop(0))
            while ydefer:
                s_matmuls(*ydefer.pop(0))

            # ---- extract diag s via row-sliced copies (no masking needed) ----
            sT = wpool.tile([32, NPAIR, BL], F32, tag="sT")
            for p in range(4):
                nc.scalar.copy(sT[0:16, p],
                               psS8[32 * p:32 * p + 16, 128 * p:128 * p + 64])
                nc.scalar.copy(sT[16:32, p],
                               psS8[32 * p + 16:32 * p + 32, 128 * p + 64:128 * p + 128])
            nc.scalar.copy(sT[0:16, 4], psS9[0:16, 0:64])
            nc.scalar.copy(sT[16:32, 4], psS9[16:32, 64:128])
            vnew = _squash(nc, wpool, sT[:], [32, NPAIR, BL], "sqi")
            if it == 0:
                nc.vector.tensor_tensor(vsum[:], vsum[:], vnew[:], ALU.add)
            else:
                vfinal = vnew

        # ---- output: vfinal [32=(o2,D), (p,b)] -> out[b, 2p+o2, D] ----
        out_ap = out_d.ap().rearrange("b (p o2) DD -> (o2 DD) p b", p=NPAIR, o2=2)
        for p in range(NPAIR):
            nc.sync.dma_start(out_ap[:, p], vfinal[:, p])


def build_program():
    nc = bacc.Bacc("TRN2", debug=False, target_bir_lowering=False)
    x3_d = nc.dram_tensor("x3", [128, NQ, d, BL], BF, kind="ExternalInput")
    xdi_d = nc.dram_tensor("xdi", [128, d, I], BF, kind="ExternalInput")
    w2_d = nc.dram_tensor("w2", [32, NPAIR, d, I], BF, kind="ExternalInput")
    w1s_d = nc.dram_tensor("w1s", [128, NT, 160], BF, kind="ExternalInput")
    idb_d = nc.dram_tensor("idb", [128, 128], BF, kind="ExternalInput")
    msk_d = nc.dram_tensor("msk", [128, 2], F32, kind="ExternalInput")
    id32_d = nc.dram_tensor("id32", [64, 64], F32, kind="ExternalInput")
    out_d = nc.dram_tensor("out", [BL, O, D], F32, kind="ExternalOutput")
    with TileContext(nc) as tc:
        _body(nc, tc, x3_d, xdi_d, w2_d, w1s_d, idb_d, id32_d, msk_d, out_d)
    nc.compile()
    return nc


def host_prep_w(W):
    """W: [1,10,1152,16,8] fp32 -> (w2, w1s, idb, id32, msk) arrays."""
    Wb = W[0].astype(bf16)
    w2 = np.ascontiguousarray(
        Wb.reshape(5, 2, I, D, d).transpose(1, 3, 0, 4, 2)).reshape(32, NPAIR, d, I)
    w1s = np.ascontiguousarray(
        Wb.reshape(5, 2, NQ, 128, D, d).transpose(3, 2, 5, 0, 1, 4)).reshape(128, NT, 160)
    idb = np.eye(128, dtype=bf16)
    id32 = np.eye(64, dtype=np.float32)
    msk = np.zeros((128, 2), np.float32)
    msk[:, 0] = np.tile(np.r_[np.ones(16), np.zeros(16)], 4)
    msk[:, 1] = 1.0 - msk[:, 0]
    return w2, w1s, idb, id32, msk


def host_prep_x(xc):
    """xc: [64, 1152, 8] fp32 -> (x3, xdi)."""
    xb = xc.astype(bf16)
    x3 = np.ascontiguousarray(xb.reshape(BL, NQ, 128, d).transpose(2, 1, 3, 0))
    xd = np.ascontiguousarray(xb.transpose(0, 2, 1))        # [64, 8, 1152]
    xdi = np.concatenate([xd, xd], axis=0)                  # [128, 8, 1152]
    return x3, xdi


_NC_CACHE = {}


def _get_nc():
    if "nc" not in _NC_CACHE:
        _NC_CACHE["nc"] = build_program()
    return _NC_CACHE["nc"]


def kernel(x, W):
    x = np.asarray(x, dtype=np.float32)
    W = np.asarray(W, dtype=np.float32)
    w2, w1s, idb, id32, msk = host_prep_w(W)
    in_maps = []
    for core in range(8):
        x3, xdi = host_prep_x(x[core * BL:(core + 1) * BL])
        in_maps.append({"x3": x3, "xdi": xdi, "w2": w2, "w1s": w1s,
                        "idb": idb, "id32": id32, "msk": msk})
    nc = _get_nc()
    res = run_bass_kernel_spmd(nc, in_maps, list(range(8)))
    out = np.concatenate([res.results[i]["out"] for i in range(8)], axis=0)
    return out.astype(np.float32)
